# revision 12
# baseline (speedup 1.0000x reference)
"""Trainium2 Bass kernel for the shifted-slice-copy stereo cost volume.

Reference semantics (B=2, C=32, H=128, W=240, D=max_disp//4=48):
    out[:, :C,  d, :, w] = left[:, :, :, w]      if w >= d else 0
    out[:, C:,  d, :, w] = right[:, :, :, w - d] if w >= d else 0
    out shape [B, 2C, D, H, W] float32  (~755 MB)

This is pure data movement (memory-regime): the kernel loads each core's
input shard into SBUF once and then streams 2*D strided DMA writes (one
per disparity per half) to the output. The invalid (w < d) prefix is never
written: run_bass_kernel_spmd pre-zeros / donates zero-filled
ExternalOutput buffers, so the masked region is already zero.

Sharding: 8 cores = 2 batches x 4 channel-blocks of 8 channels. Every
core runs the identical program over all 48 disparities for its 8
channels of both halves, so the SPMD program is uniform across cores.
"""

import sys

import numpy as np

for _p in ("/opt/trn_rl_repo",):
    if _p not in sys.path:
        sys.path.insert(0, _p)

import concourse.bass as bass
from concourse import mybir
from concourse.bass_utils import run_bass_kernel_spmd

B, C, H, W = 2, 32, 128, 240
D = 48          # max_disp // 4
CPC = 8         # channels per core (C / 4 channel-blocks)
NCORES = 8
HL = 8          # h-rows packed per partition (SBUF layout: [(c,hh)][hl][w])
HH = H // HL    # 16 -> CPC*HH = 128 partitions
PAD = D         # zero prefix elems per row-block of the padded right buffer

# Disparities d < D0_FULL write full rows (zero prefix included) from staged
# SBUF buffers as one 7680B descriptor per partition; d >= D0_FULL write only
# the valid w>=d suffix (per-row descriptors, output pre-zeroed). KBUF staging
# buffers rotate per half.
D0_FULL = 24
KBUF = 4

_NC_CACHE = None


def _build_bass():
    """One core's program: [CPC,H,W] left/right shard -> [2*CPC,D,H,W] out."""
    nc = bass.Bass()
    f32 = mybir.dt.float32
    left_c = nc.declare_dram_parameter("left_c", [CPC, H, W], f32, isOutput=False)
    right_c = nc.declare_dram_parameter("right_c", [CPC, H, W], f32, isOutput=False)
    out_c = nc.declare_dram_parameter("out_c", [2 * CPC, D, H, W], f32, isOutput=True)

    from contextlib import ExitStack

    K, D0 = KBUF, D0_FULL
    n_cop = min(K, D0)  # initial left staging copies

    with ExitStack() as ctx:
        lsb = ctx.enter_context(nc.sbuf_tensor("lsb", [CPC * HH, HL * W], f32))
        rsb = ctx.enter_context(
            nc.sbuf_tensor("rsb", [CPC * HH, HL * (PAD + W)], f32)
        )
        st_l = [
            ctx.enter_context(nc.sbuf_tensor(f"stl{k}", [CPC * HH, HL * W], f32))
            for k in range(n_cop)
        ]
        st_r = [
            ctx.enter_context(nc.sbuf_tensor(f"str{k}", [CPC * HH, HL * W], f32))
            for k in range(n_cop)
        ]
        # Sound DMA-completion tracking: a `wait sem >= 16*n` only proves the
        # first n DMAs finished when n == ALL DMAs ever issued on that sem
        # (per-SDMA-engine completion counts can skew otherwise). So loads and
        # each staging buffer get dedicated semaphores.
        ld_l = ctx.enter_context(nc.semaphore("ld_l"))
        ld_r = ctx.enter_context(nc.semaphore("ld_r"))
        l_sem = ctx.enter_context(nc.semaphore("l_sem"))
        r_sem = ctx.enter_context(nc.semaphore("r_sem"))
        vl_sem = ctx.enter_context(nc.semaphore("vl_sem"))
        vr_sem = ctx.enter_context(nc.semaphore("vr_sem"))
        bufl_sem = [
            ctx.enter_context(nc.semaphore(f"bufl{k}")) for k in range(n_cop)
        ]
        bufr_sem = [
            ctx.enter_context(nc.semaphore(f"bufr{k}")) for k in range(n_cop)
        ]
        block = ctx.enter_context(nc.Block())

        # SBUF layout: partition p = (c, hh), free = (hl, w); h = hh*HL + hl.
        # A full-row store for one (c, hh) partition is then a single
        # HL*W*4 = 7680B contiguous chunk on both sides.
        lv3 = lsb[:, :].rearrange("p (hl w) -> p hl w", hl=HL)
        rv3 = rsb[:, :].rearrange("p (hl w) -> p hl w", hl=HL)
        stl3 = [t[:, :].rearrange("p (hl w) -> p hl w", hl=HL) for t in st_l]
        str3 = [t[:, :].rearrange("p (hl w) -> p hl w", hl=HL) for t in st_r]

        # DRAM-side views keep (c, hh) as separate dims (not arithmetic-
        # mergeable); dma_start pairs them with the SBUF partition dim by
        # iteration order.
        dram_l = left_c[:, :, :].rearrange("c (hh hl) w -> c hh hl w", hl=HL)
        dram_r = right_c[:, :, :].rearrange("c (hh hl) w -> c hh hl w", hl=HL)

        def dest4(half, d):
            base = out_c[half * CPC : (half + 1) * CPC, d, :, :]
            return base.rearrange("c (hh hl) w -> c hh hl w", hl=HL)

        def dest_flat(half, d):
            base = out_c[half * CPC : (half + 1) * CPC, d, :, :]
            return base.rearrange("c (hh hl) w -> c hh (hl w)", hl=HL)

        # Streams: SP ring = left half, ACT ring = right half, DVE stages
        # full-row sources (left: rotating copies with incrementally-zeroed
        # prefixes; right: shifted-window copies out of the padded buffer).

        # Issue order per ring interleaves ungated valid-suffix stores with
        # staging-gated full-row stores so the SDMA queues never starve while
        # the staging chain (DVE copy/memset -> store -> buffer reuse) spins.
        n_valid = D - D0

        def emit_stream(eng, half, load_dst, load_src, ld_sem, v_sem, vv_sem,
                        v_thresh, full_src, valid_srcs, buf_sems):
            eng.dma_start(load_dst, load_src).then_inc(ld_sem, 16)
            eng.wait_ge(ld_sem, 16)
            order = []
            for i in range(max(D0, n_valid)):
                if i < n_valid:
                    order.append(("v", D0 + i))
                if i < D0:
                    order.append(("f", i))
            uses = [0] * max(n_cop, 1)
            nv = 0
            for kind, d in order:
                if kind == "v":
                    eng.dma_start(*valid_srcs(d)).then_inc(v_sem, 16)
                    nv += 1
                else:
                    eng.wait_ge(vv_sem, v_thresh(d))
                    eng.dma_start(dest_flat(half, d), full_src(d)).then_inc(
                        buf_sems[d % K], 16
                    )
                    uses[d % K] += 1
            eng.wait_ge(v_sem, 16 * nv)
            for k in range(n_cop):
                if uses[k]:
                    eng.wait_ge(buf_sems[k], 16 * uses[k])

        @block.sync
        def _(sync):
            emit_stream(
                sync, 0, lv3, dram_l, ld_l, l_sem, vl_sem,
                lambda d: 1 if d == 0 else n_cop + d,
                lambda d: st_l[d % K][:, :],
                lambda d: (dest4(0, d)[:, :, :, d:W], lv3[:, :, d:W]),
                bufl_sem,
            )

        @block.scalar
        def _(scalar):
            emit_stream(
                scalar, 1, rv3[:, :, PAD : PAD + W], dram_r, ld_r, r_sem,
                vr_sem,
                lambda d: d + 1,
                lambda d: st_r[d % K][:, :],
                lambda d: (dest4(1, d)[:, :, :, d:W], rv3[:, :, PAD : PAD + W - d]),
                bufr_sem,
            )

        if D0 > 0:

            @block.vector
            def _(vector):
                vector.memset(rv3[:, :, 0:PAD], 0.0)
                vector.wait_ge(ld_l, 16)
                for k in range(n_cop):
                    vector.tensor_copy(st_l[k][:, :], lsb[:, :]).then_inc(
                        vl_sem, 1
                    )
                vector.wait_ge(ld_r, 16)
                for d in range(D0):
                    if d >= 1:
                        # st_l[d%K] was last read by left full store d-K; zero
                        # the columns that newly entered the w<d prefix.
                        if d - K >= 0:
                            vector.wait_ge(bufl_sem[d % K], 16 * (d // K))
                        lo = max(0, d - K)
                        vector.memset(stl3[d % K][:, :, lo:d], 0.0).then_inc(
                            vl_sem, 1
                        )
                    # st_r[d%K] was last read by right full store d-K.
                    if d - K >= 0:
                        vector.wait_ge(bufr_sem[d % K], 16 * (d // K))
                    vector.tensor_copy(
                        str3[d % K][:, :, :], rv3[:, :, PAD - d : PAD - d + W]
                    ).then_inc(vr_sem, 1)

    return nc


def _get_nc():
    global _NC_CACHE
    if _NC_CACHE is None:
        _NC_CACHE = _build_bass()
    return _NC_CACHE


def _shard_inputs(left, right):
    in_maps = []
    for i in range(NCORES):
        b, blk = divmod(i, 4)
        c0 = blk * CPC
        in_maps.append(
            {
                "left_c": np.ascontiguousarray(left[b, c0 : c0 + CPC]),
                "right_c": np.ascontiguousarray(right[b, c0 : c0 + CPC]),
            }
        )
    return in_maps


def _gather_outputs(results):
    out = np.empty((B, 2 * C, D, H, W), np.float32)
    for i in range(NCORES):
        b, blk = divmod(i, 4)
        c0 = blk * CPC
        oc = results[i]["out_c"]
        out[b, c0 : c0 + CPC] = oc[:CPC]
        out[b, C + c0 : C + c0 + CPC] = oc[CPC:]
    return out


def run_sharded(left, right, **run_kwargs):
    """Compile+run the SPMD kernel; returns (full_output, BassKernelResults)."""
    res = run_bass_kernel_spmd(
        _get_nc(), _shard_inputs(left, right), list(range(NCORES)), **run_kwargs
    )
    return _gather_outputs(res.results), res


def kernel(**inputs):
    left = np.asarray(inputs["left_feature"], dtype=np.float32)
    right = np.asarray(inputs["right_feature"], dtype=np.float32)
    max_disp = int(np.asarray(inputs["max_disp"]))
    assert left.shape == (B, C, H, W), left.shape
    assert right.shape == (B, C, H, W), right.shape
    assert max_disp // 4 == D, max_disp
    out, _ = run_sharded(left, right)
    return out
